# revision 25
# baseline (speedup 1.0000x reference)
"""Trainium2 Bass kernel for nn_MixtureOfRoutingAttention.

Strategy: data-parallel over B=8 (one sample per NeuronCore). The top-1
routing argmax is computed on host (it only decides dispatch); the selected
expert weight stacks are gathered per sample on host, packed into large
contiguous bf16 blocks, and shipped pre-transposed to each core. Everything
x-dependent (LayerNorms, QKV/out projections, three attentions, MLP) runs on
device with bf16 matmul operands and fp32 PSUM accumulation.

Device-side layout: activations are kept feature-major (x^T: [D, T], feature
on partitions) so every GEMM contracts over the partition axis with zero
on-chip transposes. Attention uses transposed scores S^T[j, i] so softmax
normalization is deferred: a ones-column prepended to V yields the softmax
denominators as row 0 of the PV psum accumulation (partition 0, so it feeds
partition_broadcast directly). The temporal branch computes only the lower
triangle of S^T and the matching PV blocks. LayerNorm rstd is computed as
exp(-0.5*ln(var+eps)) so the whole kernel needs only the natural_log_exp
and gelu ACT table sets (2 table loads per iteration instead of 4).
"""

import math
import os
from contextlib import ExitStack

import numpy as np
import ml_dtypes

import concourse.bass as bass
import concourse.bacc as bacc
import concourse.tile as tile
import concourse.mybir as mybir
from concourse import bass_utils

dt = mybir.dt
AF = mybir.ActivationFunctionType
ALU = mybir.AluOpType

P = 128
T = 512
D = 768
H = 8
HD = 96
DFF = 3072
NCORES = 8
ND = D // P  # 6
NT = T // P  # 4
NF = DFF // P  # 24
FCC = 4  # mlp fc-chunk (DMA granularity)
SCALE = 1.0 / math.sqrt(HD)
EPS = 1e-5
F32 = dt.float32
BF = dt.bfloat16
NPBF = ml_dtypes.bfloat16


def _bcast_ap(src_1d, parts=P):
    """Partition-broadcast DMA access pattern for a 1-D DRAM AP."""
    return bass.AP(
        tensor=src_1d.tensor,
        offset=src_1d.offset,
        ap=[[0, parts], list(src_1d.ap[0])],
    )


PHASE_MARKS = []


def build(repeat: int = 1):
    PHASE_MARKS.clear()
    nc = bacc.Bacc(
        "TRN2",
        target_bir_lowering=False,
        debug=False,
        enable_asserts=False,
        num_devices=NCORES,
    )

    def din(name, shape, dtype=BF):
        return nc.dram_tensor(name, shape, dtype, kind="ExternalInput").ap()

    xT_d = din("xT", [D, T])
    diag_d = din("diag", [P, P])
    ln_d = {n: din(n, [D], F32) for n in ("g_s", "b_s", "g_t", "b_t", "g_m", "b_m")}

    wqk_d = {b: din(f"{b}Wqk", [P, 2 * H * ND * HD]) for b in ("sp", "tp", "cx")}
    wv_d = {b: din(f"{b}Wv", [P, ND * D]) for b in ("sp", "tp", "cx")}
    wo_d = {b: din(f"{b}Wo", [HD, ND * H * P]) for b in ("sp", "tp", "cx")}
    bq_d = {b: din(f"{b}Bq", [D], F32) for b in ("sp", "tp", "cx")}
    bk_d = {b: din(f"{b}Bk", [D], F32) for b in ("sp", "tp", "cx")}
    bv_d = {b: din(f"{b}Bv", [D], F32) for b in ("sp", "tp", "cx")}
    bo_d = {b: din(f"{b}Bo", [D], F32) for b in ("sp", "tp", "cx")}

    mW1_d = din("mW1", [P, NF * ND * P])
    mB1_d = din("mB1", [DFF], F32)
    mW2_d = din("mW2", [P, NF * D])
    mB2_d = din("mB2", [D], F32)

    outT_d = nc.dram_tensor("outT", [D, T], BF, kind="ExternalOutput").ap()

    with tile.TileContext(nc) as tc, ExitStack() as ctx:
        ctx.enter_context(
            nc.allow_low_precision(reason="bf16 matmul operands, fp32 accumulation")
        )
        const = ctx.enter_context(tc.tile_pool(name="const", bufs=1))
        big = ctx.enter_context(tc.tile_pool(name="big", bufs=1))
        wqkp = ctx.enter_context(tc.tile_pool(name="wqkp", bufs=2))
        wvp = ctx.enter_context(tc.tile_pool(name="wvp", bufs=2))
        wop = ctx.enter_context(tc.tile_pool(name="wop", bufs=1))
        w1p = ctx.enter_context(tc.tile_pool(name="w1p", bufs=2))
        w2p = ctx.enter_context(tc.tile_pool(name="w2p", bufs=2))
        tmp = ctx.enter_context(tc.tile_pool(name="tmp", bufs=4))
        ex = ctx.enter_context(tc.tile_pool(name="ex", bufs=8))
        qk = ctx.enter_context(tc.tile_pool(name="qk", bufs=8))
        rows = ctx.enter_context(tc.tile_pool(name="rows", bufs=6))
        ps = ctx.enter_context(tc.tile_pool(name="ps", bufs=8, space="PSUM"))

        def pst(nm):
            return ps.tile([P, T], F32, name=nm, tag="ps", bufs=8)

        # ---- constants / params (loaded once, outside any repeat loop) ----
        ones_bf = const.tile([P, 1], BF, name="ones_bf", tag="ones")
        nc.vector.memset(ones_bf, 1.0)
        diag_sb = const.tile([P, P], BF, name="diag_sb", tag="diag")
        nc.sync.dma_start(out=diag_sb, in_=diag_d)
        epsc = const.tile([P, 1], F32, name="epsc", tag="eps")
        nc.vector.memset(epsc, EPS)

        ln_sb = {}
        for n in ln_d:
            t = const.tile([P, ND], F32, name=f"ln_{n}", tag=f"ln_{n}")
            nc.sync.dma_start(out=t, in_=ln_d[n].rearrange("(a p) -> p a", p=P))
            ln_sb[n] = t

        bq96, bk96, vbias, bo_sb = {}, {}, {}, {}
        for b in ("sp", "tp", "cx"):
            t = const.tile([HD, H], F32, name=f"bq96_{b}", tag=f"bq96_{b}")
            nc.sync.dma_start(out=t, in_=bq_d[b].rearrange("(h k) -> k h", k=HD))
            bq96[b] = t
            t = const.tile([HD, H], F32, name=f"bk96_{b}", tag=f"bk96_{b}")
            nc.sync.dma_start(out=t, in_=bk_d[b].rearrange("(h k) -> k h", k=HD))
            bk96[b] = t
            t = const.tile([P, D], F32, name=f"vb_{b}", tag=f"vb_{b}")
            nc.gpsimd.dma_start(out=t, in_=_bcast_ap(bv_d[b]))
            vbias[b] = t
            t = const.tile([P, ND], F32, name=f"bo_{b}", tag=f"bo_{b}")
            nc.sync.dma_start(out=t, in_=bo_d[b].rearrange("(a p) -> p a", p=P))
            bo_sb[b] = t

        mB1_sb = const.tile([P, NF], F32, name="mB1_sb", tag="mB1")
        nc.sync.dma_start(out=mB1_sb, in_=mB1_d.rearrange("(a p) -> p a", p=P))
        mB2_sb = const.tile([P, ND], F32, name="mB2_sb", tag="mB2")
        nc.sync.dma_start(out=mB2_sb, in_=mB2_d.rearrange("(a p) -> p a", p=P))

        # ---- body helpers ----

        def ln_stats(src):
            """src: [P, ND, T] bf16. Returns (meanb, rstdb) [P, T] bf16."""
            ps_m = pst("ps_m")
            ps_s = pst("ps_s")
            for a in range(ND):
                sq = tmp.tile([P, T], BF, name="sq", tag="tmp")
                nc.scalar.square(sq, src[:, a, :])
                nc.tensor.matmul(
                    ps_m[0:1, :], ones_bf, src[:, a, :],
                    start=(a == 0), stop=(a == ND - 1),
                )
                nc.tensor.matmul(
                    ps_s[0:1, :], ones_bf, sq,
                    start=(a == 0), stop=(a == ND - 1),
                )
            mrow = rows.tile([1, T], F32, name="mrow", tag="rows")
            nc.vector.tensor_scalar_mul(mrow, ps_m[0:1, :], 1.0 / D)
            srow = rows.tile([1, T], F32, name="srow", tag="rows")
            nc.vector.tensor_scalar_mul(srow, ps_s[0:1, :], 1.0 / D)
            trow = rows.tile([1, T], F32, name="trow", tag="rows")
            nc.vector.tensor_mul(trow, mrow, mrow)
            # var = E[x^2] - m^2 ; rstd = exp(-0.5*ln(var+eps))
            nc.vector.tensor_sub(srow, srow, trow)
            urow = rows.tile([1, T], F32, name="urow", tag="rows")
            nc.scalar.activation(urow, srow, AF.Ln, bias=epsc[0:1, :])
            rrow = rows.tile([1, T], BF, name="rrow", tag="rows")
            nc.scalar.activation(rrow, urow, AF.Exp, scale=-0.5)
            mrow_bf = rows.tile([1, T], BF, name="mrow_bf", tag="rows")
            nc.vector.tensor_copy(mrow_bf, mrow)

            meanb = big.tile([P, T], BF, name="meanb", tag="meanb", bufs=2)
            nc.gpsimd.partition_broadcast(meanb, mrow_bf)
            rstdb = big.tile([P, T], BF, name="rstdb", tag="rstdb", bufs=2)
            nc.gpsimd.partition_broadcast(rstdb, rrow)
            return meanb, rstdb

        def ln_apply(src, meanb, rstdb, outs):
            """outs: list of (dst [P, ND, T] bf16, gamma_sb, beta_sb)."""
            for a in range(ND):
                xc = tmp.tile([P, T], BF, name="xc", tag="tmp")
                nc.vector.tensor_sub(xc, src[:, a, :], meanb)
                nc.vector.tensor_mul(xc, xc, rstdb)
                for dst, g_sb, b_sb in outs:
                    nc.scalar.activation(
                        dst[:, a, :], xc, AF.Identity,
                        bias=b_sb[:, a : a + 1], scale=g_sb[:, a : a + 1],
                    )

        def gemm_head(src, wqk_sb, qki, bias96, h, dst96, use_act=False):
            """dst96[0:HD, :] = (W[:, head h cols].T @ src) + bias."""
            pq = pst("pq")
            for a in range(ND):
                nc.tensor.matmul(
                    pq[0:HD, :], wqk_sb[:, qki, h, a, :], src[:, a, :],
                    start=(a == 0), stop=(a == ND - 1),
                )
            if use_act:
                nc.scalar.activation(
                    dst96[0:HD, :], pq[0:HD, :], AF.Identity,
                    bias=bias96[:, h : h + 1],
                )
            else:
                nc.vector.tensor_scalar_add(
                    dst96[0:HD, :], pq[0:HD, :], bias96[:, h : h + 1]
                )

        def gemm_v_token(src, wv_sb, vbias_bc, Vt):
            """Vt: [P, NT, H, HD+1] token-major V with trailing ones column."""
            nc.vector.memset(Vt[:, :, :, HD], 1.0)
            for half, n in ((0, 512), (1, 256)):
                pvs = [pst(f"pv{t}") for t in range(NT)]
                for a in range(ND):
                    for t in range(NT):
                        nc.tensor.matmul(
                            pvs[t][:, 0:n],
                            src[:, a, t * P : (t + 1) * P],
                            wv_sb[:, a, half * 512 : half * 512 + n],
                            start=(a == 0), stop=(a == ND - 1),
                        )
                for t in range(NT):
                    if half == 0:
                        nc.vector.tensor_add(
                            Vt[:, t, 0:5, 0:HD],
                            pvs[t][:, 0:480].rearrange("p (h k) -> p h k", k=HD),
                            vbias_bc[:, 0:480].rearrange("p (h k) -> p h k", k=HD),
                        )
                        nc.vector.tensor_add(
                            Vt[:, t, 5, 0:32],
                            pvs[t][:, 480:512],
                            vbias_bc[:, 480:512],
                        )
                    else:
                        nc.vector.tensor_add(
                            Vt[:, t, 5, 32:HD],
                            pvs[t][:, 0:64],
                            vbias_bc[:, 512:576],
                        )
                        nc.vector.tensor_add(
                            Vt[:, t, 6:8, 0:HD],
                            pvs[t][:, 64:256].rearrange("p (h k) -> p h k", k=HD),
                            vbias_bc[:, 576:768].rearrange("p (h k) -> p h k", k=HD),
                        )

        def attn_head_core(qh, kh, Vt, attnT, causal, h):
            """Scores, exp, PV, deferred-softmax normalization for one head."""
            ets = []
            for jc in range(NT):
                i0 = jc * P if causal else 0
                pS = pst("pS")
                nc.tensor.matmul(
                    pS[:, 0 : T - i0],
                    kh[0:HD, jc * P : (jc + 1) * P],
                    qh[0:HD, i0:T],
                    start=True, stop=True,
                )
                et = ex.tile([P, T], BF, name="et", tag="ex")
                nc.scalar.activation(
                    et[:, i0:T], pS[:, 0 : T - i0], AF.Exp, scale=SCALE
                )
                if causal:
                    nc.gpsimd.tensor_mul(
                        et[:, i0 : i0 + P], et[:, i0 : i0 + P], diag_sb
                    )
                ets.append(et)
            pa = pst("pa")
            for jc in range(NT):
                i0 = jc * P if causal else 0
                # jc=0 covers the full psum row (start lazily zeroes the whole
                # 2KB zero region); later jc's accumulate only their causal
                # suffix [jc*P:T].
                nc.tensor.matmul(
                    pa[0 : HD + 1, i0:T], Vt[:, jc, h, :], ets[jc][:, i0:T],
                    start=(jc == 0), stop=(jc == NT - 1),
                )
            srow = rows.tile([HD + 1, T], BF, name="sumrow", tag="srow", bufs=3)
            nc.vector.reciprocal(srow[HD : HD + 1, :], pa[HD : HD + 1, :])
            # Replicating SBUF->SBUF DMA: broadcast the reciprocal row
            # (partition HD) to all partitions via a stride-0 middle dim.
            s = srow[HD : HD + 1, :]
            rbc = tmp.tile([P, T], BF, name="rbc", tag="tmp")
            nc.sync.dma_start(
                out=rbc,
                in_=bass.AP(
                    tensor=s.tensor, offset=s.offset,
                    ap=[list(s.ap[0]), [0, P], list(s.ap[-1])],
                ),
            )
            nc.vector.tensor_mul(
                attnT[0:HD, h, :], pa[0:HD, :], rbc[0:HD, :]
            )

        def attn_branch(src_q, src_k, Vt, attnT, causal, wqk_sb, bq, bk):
            """Per-head q/k projection software-pipelined with attention."""
            qs, ks = [None] * H, [None] * H
            for h in range(H):
                qs[h] = qk.tile([P, T], BF, name="qh", tag="qk")
                ks[h] = qk.tile([P, T], BF, name="kh", tag="qk")
                gemm_head(src_q, wqk_sb, 0, bq, h, qs[h], use_act=True)
                gemm_head(src_k, wqk_sb, 1, bk, h, ks[h], use_act=False)
                if h > 0:
                    attn_head_core(qs[h - 1], ks[h - 1], Vt, attnT, causal, h - 1)
                    qs[h - 1] = ks[h - 1] = None
            attn_head_core(qs[H - 1], ks[H - 1], Vt, attnT, causal, H - 1)

        def attn_two_branches(a, b):
            """Interleave two independent branches head-by-head: each
            branch's projections fill the other's exp-wait PE bubbles."""
            (src_qa, src_ka, Vta, attnTa, causala, wqka, bqa, bka) = a
            (src_qb, src_kb, Vtb, attnTb, causalb, wqkb, bqb, bkb) = b
            qa, ka = [None] * H, [None] * H
            qb, kb = [None] * H, [None] * H
            for h in range(H):
                qa[h] = qk.tile([P, T], BF, name="qha", tag="qk")
                ka[h] = qk.tile([P, T], BF, name="kha", tag="qk")
                gemm_head(src_qa, wqka, 0, bqa, h, qa[h], use_act=True)
                gemm_head(src_ka, wqka, 1, bka, h, ka[h], use_act=False)
                if h > 0:
                    attn_head_core(
                        qb[h - 1], kb[h - 1], Vtb, attnTb, causalb, h - 1
                    )
                    qb[h - 1] = kb[h - 1] = None
                qb[h] = qk.tile([P, T], BF, name="qhb", tag="qk")
                kb[h] = qk.tile([P, T], BF, name="khb", tag="qk")
                gemm_head(src_qb, wqkb, 0, bqb, h, qb[h], use_act=True)
                gemm_head(src_kb, wqkb, 1, bkb, h, kb[h], use_act=False)
                attn_head_core(qa[h], ka[h], Vta, attnTa, causala, h)
                qa[h] = ka[h] = None
            attn_head_core(qb[H - 1], kb[H - 1], Vtb, attnTb, causalb, H - 1)

        def out_proj(attnT, wo_sb, bias_sb, dst, residual=None):
            for e in range(ND):
                po = pst("po")
                for h in range(H):
                    nc.tensor.matmul(
                        po, wo_sb[:, e, h, :], attnT[0:HD, h, :],
                        start=(h == 0), stop=(h == H - 1),
                    )
                if residual is None:
                    nc.vector.tensor_scalar_add(
                        dst[:, e, :], po, bias_sb[:, e : e + 1]
                    )
                else:
                    nc.vector.scalar_tensor_tensor(
                        dst[:, e, :], po, bias_sb[:, e : e + 1],
                        residual[:, e, :], ALU.add, ALU.add,
                    )

        def load_wqk(b):
            t = wqkp.tile([P, 2, H, ND, HD], BF, name=f"wqk_{b}", tag="wqk")
            nc.sync.dma_start(
                out=t,
                in_=wqk_d[b].rearrange("p (q h a k) -> p q h a k", q=2, h=H, a=ND),
            )
            return t

        def load_wv(b):
            t = wvp.tile([P, ND, D], BF, name=f"wv_{b}", tag="wv")
            nc.sync.dma_start(
                out=t, in_=wv_d[b].rearrange("p (a e) -> p a e", a=ND)
            )
            return t

        def load_wo(b):
            t = wop.tile([HD, ND, H, P], BF, name=f"wo_{b}", tag="wo")
            nc.sync.dma_start(
                out=t, in_=wo_d[b].rearrange("k (e h ec) -> k e h ec", e=ND, h=H)
            )
            return t

        def mlp(xn3, x1T, outT_sb):
            ph2 = [
                ps.tile([P, T], F32, name=f"h2_{e}", tag="ps", bufs=8)
                for e in range(ND)
            ]
            for c in range(NF // FCC):
                w1t = w1p.tile([P, FCC, ND, P], BF, name="w1t", tag="w1")
                nc.sync.dma_start(
                    out=w1t,
                    in_=mW1_d.rearrange("p (f a e) -> p f a e", f=NF, a=ND)[
                        :, c * FCC : (c + 1) * FCC, :, :
                    ],
                )
                w2t = w2p.tile([P, FCC, D], BF, name="w2t", tag="w2")
                nc.sync.dma_start(
                    out=w2t,
                    in_=mW2_d.rearrange("p (f e) -> p f e", f=NF)[
                        :, c * FCC : (c + 1) * FCC, :
                    ],
                )
                for j in range(FCC):
                    fc = c * FCC + j
                    ph1 = pst("ph1")
                    for a in range(ND):
                        nc.tensor.matmul(
                            ph1, w1t[:, j, a, :], xn3[:, a, :],
                            start=(a == 0), stop=(a == ND - 1),
                        )
                    yt = tmp.tile([P, T], BF, name="yt", tag="tmp")
                    nc.scalar.activation(
                        yt, ph1, AF.Gelu, bias=mB1_sb[:, fc : fc + 1]
                    )
                    for e in range(ND):
                        nc.tensor.matmul(
                            ph2[e], w2t[:, j, e * P : (e + 1) * P], yt,
                            start=(fc == 0), stop=(fc == NF - 1),
                        )
            for e in range(ND):
                nc.vector.scalar_tensor_tensor(
                    outT_sb[:, e, :], ph2[e], mB2_sb[:, e : e + 1],
                    x1T[:, e, :], ALU.add, ALU.add,
                )

        def _mark(phase):
            PHASE_MARKS.append((phase, int(nc.get_next_instruction_name()[2:])))

        def body():
            _mark("load_x")
            xT_sb = big.tile([P, ND, T], BF, name="xT_sb", tag="xT")
            nc.sync.dma_start(
                out=xT_sb, in_=xT_d.rearrange("(a p) t -> p a t", p=P)
            )
            _mark("ln0")
            meanb, rstdb = ln_stats(xT_sb)
            xn_s = big.tile([P, ND, T], BF, name="xn_s", tag="xn_s")
            xn_t = big.tile([P, ND, T], BF, name="xn_t", tag="xn_t")
            ln_apply(
                xT_sb, meanb, rstdb,
                [
                    (xn_s, ln_sb["g_s"], ln_sb["b_s"]),
                    (xn_t, ln_sb["g_t"], ln_sb["b_t"]),
                ],
            )

            # --- spatial branch (temporal V is emitted early for overlap) ---
            _mark("sp_v")
            wv_sp = load_wv("sp")
            Vt = big.tile([P, NT, H, HD + 1], BF, name="Vt_s", tag="Vt", bufs=2)
            gemm_v_token(xn_s, wv_sp, vbias["sp"], Vt)
            _mark("tp_v")
            wv_tp = load_wv("tp")
            Vt2 = big.tile([P, NT, H, HD + 1], BF, name="Vt_t", tag="Vt", bufs=2)
            gemm_v_token(xn_t, wv_tp, vbias["tp"], Vt2)
            attnT = big.tile([P, H, T], BF, name="attnT_s", tag="attnT", bufs=2)
            attnT2 = big.tile([P, H, T], BF, name="attnT_t", tag="attnT", bufs=2)
            _mark("sp_attn")
            wqk_sp = load_wqk("sp")
            wqk_tp = load_wqk("tp")
            attn_two_branches(
                (xn_s, xn_s, Vt, attnT, False, wqk_sp, bq96["sp"], bk96["sp"]),
                (xn_t, xn_t, Vt2, attnT2, True, wqk_tp, bq96["tp"], bk96["tp"]),
            )
            _mark("sp_oproj")
            wo_sp = load_wo("sp")
            soT = big.tile([P, ND, T], BF, name="soT", tag="soT")
            out_proj(attnT, wo_sp, bo_sb["sp"], soT)

            # --- temporal branch ---
            _mark("tp_oproj")
            wo_tp = load_wo("tp")
            toT = big.tile([P, ND, T], BF, name="toT", tag="toT")
            out_proj(attnT2, wo_tp, bo_sb["tp"], toT, residual=xn_t)

            # --- cross attention ---
            _mark("cx_v")
            wv_cx = load_wv("cx")
            Vt3 = big.tile([P, NT, H, HD + 1], BF, name="Vt_c", tag="Vt", bufs=2)
            gemm_v_token(toT, wv_cx, vbias["cx"], Vt3)
            attnT3 = big.tile([P, H, T], BF, name="attnT_c", tag="attnT", bufs=2)
            _mark("cx_attn")
            wqk_cx = load_wqk("cx")
            attn_branch(soT, toT, Vt3, attnT3, False, wqk_cx, bq96["cx"], bk96["cx"])
            _mark("cx_oproj")
            wo_cx = load_wo("cx")
            x1T = big.tile([P, ND, T], BF, name="x1T", tag="x1T")
            out_proj(attnT3, wo_cx, bo_sb["cx"], x1T, residual=xT_sb)

            # --- MLP ---
            _mark("ln3")
            meanb3, rstdb3 = ln_stats(x1T)
            xn3 = big.tile([P, ND, T], BF, name="xn3", tag="xn_s")
            ln_apply(x1T, meanb3, rstdb3, [(xn3, ln_sb["g_m"], ln_sb["b_m"])])
            outT_sb = big.tile([P, ND, T], BF, name="outT_sb", tag="toT")
            _mark("mlp")
            mlp(xn3, x1T, outT_sb)
            nc.sync.dma_start(
                out=outT_d.rearrange("(a p) t -> p a t", p=P), in_=outT_sb
            )

        if repeat == 1:
            body()
        else:
            with tc.For_i(0, repeat, 1):
                body()

    nc.compile()
    return nc


def _route(inputs):
    """Top-1 expert indices per sample, computed exactly as the reference
    (jax on CPU, f32) — softmax is monotonic so argmax of logits suffices."""
    import jax
    import jax.numpy as jnp

    cpu = jax.devices("cpu")[0]
    with jax.default_device(cpu):
        x = jnp.asarray(inputs["x"])
        h = jax.nn.gelu(
            x.mean(1) @ jnp.asarray(inputs["router_w1"]).T
            + jnp.asarray(inputs["router_b1"]),
            approximate=False,
        )
        logits = (
            h @ jnp.asarray(inputs["router_w2"]).T + jnp.asarray(inputs["router_b2"])
        )
        logits = np.asarray(logits)
    K = logits.shape[1] // 2
    idx_s = np.argmax(logits[:, :K], axis=-1)
    idx_t = np.argmax(logits[:, K:], axis=-1)
    return idx_s, idx_t


_cache = {}


def _get_nc(repeat=1):
    key = ("nc", repeat)
    if key not in _cache:
        _cache[key] = build(repeat=repeat)
    return _cache[key]


def _f(a):
    return np.ascontiguousarray(np.asarray(a), dtype=np.float32)


def _bf(a):
    return np.ascontiguousarray(np.asarray(a, dtype=np.float32).astype(NPBF))


def _pack_qk_pair(wqT, wkT):
    # wqT/wkT: [D, D] = W^T columns (d, e); e = h*HD+k.
    # -> [P, 2*H*ND*HD] so the whole q/k weight pair is one contiguous DMA.
    arr = np.stack([np.asarray(wqT), np.asarray(wkT)])  # [2, D, D]
    arr = arr.reshape(2, ND, P, H, HD).transpose(2, 0, 3, 1, 4)
    return _bf(arr.reshape(P, 2 * H * ND * HD))


def _pack_v(wT):
    # [D, D] (d, e) -> [P, ND*D]
    return _bf(np.asarray(wT).reshape(ND, P, D).transpose(1, 0, 2).reshape(P, ND * D))


def _pack_wo(w):
    # w: [D, D] (e, d) -> W^T[d, e], d = h*HD+k -> [HD, ND*H*P]
    wt = np.asarray(w).T.reshape(H, HD, ND, P)
    return _bf(wt.transpose(1, 2, 0, 3).reshape(HD, ND * H * P))


def _pack_w1(w1):
    # w1: [DFF, D] -> W1^T [D, DFF] -> [P, NF*ND*P]
    w1t = np.asarray(w1).T.reshape(ND, P, NF, P)
    return _bf(w1t.transpose(1, 2, 0, 3).reshape(P, NF * ND * P))


def _pack_w2(w2):
    # w2: [D, DFF] -> W2^T [DFF, D] -> [P, NF*D]
    w2t = np.asarray(w2).T.reshape(NF, P, D)
    return _bf(w2t.transpose(1, 0, 2).reshape(P, NF * D))


def make_in_maps(inputs):
    idx_s, idx_t = _route(inputs)
    diag = np.triu(np.ones((P, P), dtype=np.float32))  # 1 where p <= q
    cWqkvT = np.asarray(inputs["cross_wqkv"]).astype(np.float32).T
    cb = _f(inputs["cross_bqkv"])
    shared = dict(
        diag=_bf(diag),
        g_s=_f(inputs["norm_s_g"]),
        b_s=_f(inputs["norm_s_b"]),
        g_t=_f(inputs["norm_t_g"]),
        b_t=_f(inputs["norm_t_b"]),
        g_m=_f(inputs["norm_mlp_g"]),
        b_m=_f(inputs["norm_mlp_b"]),
        cxWqk=_pack_qk_pair(cWqkvT[:, 0:D], cWqkvT[:, D : 2 * D]),
        cxWv=_pack_v(cWqkvT[:, 2 * D : 3 * D]),
        cxWo=_pack_wo(np.asarray(inputs["cross_wo"])),
        cxBq=cb[0:D],
        cxBk=cb[D : 2 * D],
        cxBv=cb[2 * D : 3 * D],
        cxBo=_f(inputs["cross_bo"]),
        mW1=_pack_w1(np.asarray(inputs["mlp_w1"])),
        mB1=_f(inputs["mlp_b1"]),
        mW2=_pack_w2(np.asarray(inputs["mlp_w2"])),
        mB2=_f(inputs["mlp_b2"]),
    )
    x = np.asarray(inputs["x"])
    in_maps = []
    for b in range(NCORES):
        s = int(idx_s[b])
        t = int(idx_t[b])
        m = dict(shared)
        m["xT"] = _bf(x[b].T)
        spWqkvT = np.asarray(inputs["sp_wqkv"])[s].astype(np.float32).T
        spb = _f(np.asarray(inputs["sp_bqkv"])[s])
        m["spWqk"] = _pack_qk_pair(spWqkvT[:, 0:D], spWqkvT[:, D : 2 * D])
        m["spWv"] = _pack_v(spWqkvT[:, 2 * D : 3 * D])
        m["spWo"] = _pack_wo(np.asarray(inputs["sp_wo"])[s])
        m["spBq"] = spb[0:D]
        m["spBk"] = spb[D : 2 * D]
        m["spBv"] = spb[2 * D : 3 * D]
        m["spBo"] = _f(np.asarray(inputs["sp_bo"])[s])
        m["tpWqk"] = _pack_qk_pair(
            np.asarray(inputs["tp_wq"])[t].astype(np.float32).T,
            np.asarray(inputs["tp_wk"])[t].astype(np.float32).T,
        )
        m["tpWv"] = _pack_v(np.asarray(inputs["tp_wv"])[t].astype(np.float32).T)
        m["tpWo"] = _pack_wo(np.asarray(inputs["tp_wo"])[t])
        m["tpBq"] = _f(np.asarray(inputs["tp_bq"])[t])
        m["tpBk"] = _f(np.asarray(inputs["tp_bk"])[t])
        m["tpBv"] = _f(np.asarray(inputs["tp_bv"])[t])
        m["tpBo"] = _f(np.asarray(inputs["tp_bo"])[t])
        in_maps.append(m)
    return in_maps


def kernel(**inputs) -> np.ndarray:
    repeat = int(os.environ.get("KREPEAT", "1"))
    nc = _get_nc(repeat=repeat)
    in_maps = make_in_maps(inputs)
    res = bass_utils.run_bass_kernel_spmd(nc, in_maps, core_ids=list(range(NCORES)))
    out = np.stack(
        [
            np.ascontiguousarray(
                np.asarray(res.results[b]["outT"], dtype=np.float32).T
            )
            for b in range(NCORES)
        ]
    )
    return out


# revision 26
# speedup vs baseline: 1.0038x; 1.0038x over previous
"""Trainium2 Bass kernel for nn_MixtureOfRoutingAttention.

Strategy: data-parallel over B=8 (one sample per NeuronCore). The top-1
routing argmax is computed on host (it only decides dispatch); the selected
expert weight stacks are gathered per sample on host, packed into large
contiguous bf16 blocks, and shipped pre-transposed to each core. Everything
x-dependent (LayerNorms, QKV/out projections, three attentions, MLP) runs on
device with bf16 matmul operands and fp32 PSUM accumulation.

Device-side layout: activations are kept feature-major (x^T: [D, T], feature
on partitions) so every GEMM contracts over the partition axis with zero
on-chip transposes. Attention uses transposed scores S^T[j, i] so softmax
normalization is deferred: a ones-column prepended to V yields the softmax
denominators as row 0 of the PV psum accumulation (partition 0, so it feeds
partition_broadcast directly). The temporal branch computes only the lower
triangle of S^T and the matching PV blocks. LayerNorm rstd is computed as
exp(-0.5*ln(var+eps)) so the whole kernel needs only the natural_log_exp
and gelu ACT table sets (2 table loads per iteration instead of 4).
"""

import math
import os
from contextlib import ExitStack

import numpy as np
import ml_dtypes

import concourse.bass as bass
import concourse.bacc as bacc
import concourse.tile as tile
import concourse.mybir as mybir
from concourse import bass_utils

dt = mybir.dt
AF = mybir.ActivationFunctionType
ALU = mybir.AluOpType

P = 128
T = 512
D = 768
H = 8
HD = 96
DFF = 3072
NCORES = 8
ND = D // P  # 6
NT = T // P  # 4
NF = DFF // P  # 24
FCC = 4  # mlp fc-chunk (DMA granularity)
SCALE = 1.0 / math.sqrt(HD)
EPS = 1e-5
F32 = dt.float32
BF = dt.bfloat16
NPBF = ml_dtypes.bfloat16


def _bcast_ap(src_1d, parts=P):
    """Partition-broadcast DMA access pattern for a 1-D DRAM AP."""
    return bass.AP(
        tensor=src_1d.tensor,
        offset=src_1d.offset,
        ap=[[0, parts], list(src_1d.ap[0])],
    )


PHASE_MARKS = []


def build(repeat: int = 1):
    PHASE_MARKS.clear()
    nc = bacc.Bacc(
        "TRN2",
        target_bir_lowering=False,
        debug=False,
        enable_asserts=False,
        num_devices=NCORES,
    )

    def din(name, shape, dtype=BF):
        return nc.dram_tensor(name, shape, dtype, kind="ExternalInput").ap()

    xT_d = din("xT", [D, T])
    diag_d = din("diag", [P, P])
    ln_d = {n: din(n, [D], F32) for n in ("g_s", "b_s", "g_t", "b_t", "g_m", "b_m")}

    wqk_d = {b: din(f"{b}Wqk", [P, 2 * H * ND * HD]) for b in ("sp", "tp", "cx")}
    wv_d = {b: din(f"{b}Wv", [P, ND * D]) for b in ("sp", "tp", "cx")}
    wo_d = {b: din(f"{b}Wo", [HD, ND * H * P]) for b in ("sp", "tp", "cx")}
    bq_d = {b: din(f"{b}Bq", [D], F32) for b in ("sp", "tp", "cx")}
    bk_d = {b: din(f"{b}Bk", [D], F32) for b in ("sp", "tp", "cx")}
    bv_d = {b: din(f"{b}Bv", [D], F32) for b in ("sp", "tp", "cx")}
    bo_d = {b: din(f"{b}Bo", [D], F32) for b in ("sp", "tp", "cx")}

    mW1_d = din("mW1", [P, NF * ND * P])
    mB1_d = din("mB1", [DFF], F32)
    mW2_d = din("mW2", [P, NF * D])
    mB2_d = din("mB2", [D], F32)

    outT_d = nc.dram_tensor("outT", [D, T], BF, kind="ExternalOutput").ap()

    with tile.TileContext(nc) as tc, ExitStack() as ctx:
        ctx.enter_context(
            nc.allow_low_precision(reason="bf16 matmul operands, fp32 accumulation")
        )
        const = ctx.enter_context(tc.tile_pool(name="const", bufs=1))
        big = ctx.enter_context(tc.tile_pool(name="big", bufs=1))
        wqkp = ctx.enter_context(tc.tile_pool(name="wqkp", bufs=2))
        wvp = ctx.enter_context(tc.tile_pool(name="wvp", bufs=2))
        wop = ctx.enter_context(tc.tile_pool(name="wop", bufs=1))
        w1p = ctx.enter_context(tc.tile_pool(name="w1p", bufs=2))
        w2p = ctx.enter_context(tc.tile_pool(name="w2p", bufs=2))
        tmp = ctx.enter_context(tc.tile_pool(name="tmp", bufs=4))
        ex = ctx.enter_context(tc.tile_pool(name="ex", bufs=5))
        qk = ctx.enter_context(tc.tile_pool(name="qk", bufs=6))
        rows = ctx.enter_context(tc.tile_pool(name="rows", bufs=6))
        ps = ctx.enter_context(tc.tile_pool(name="ps", bufs=8, space="PSUM"))

        def pst(nm):
            return ps.tile([P, T], F32, name=nm, tag="ps", bufs=8)

        # ---- constants / params (loaded once, outside any repeat loop) ----
        ones_bf = const.tile([P, 1], BF, name="ones_bf", tag="ones")
        nc.vector.memset(ones_bf, 1.0)
        diag_sb = const.tile([P, P], BF, name="diag_sb", tag="diag")
        nc.sync.dma_start(out=diag_sb, in_=diag_d)
        epsc = const.tile([P, 1], F32, name="epsc", tag="eps")
        nc.vector.memset(epsc, EPS)

        ln_sb = {}
        for n in ln_d:
            t = const.tile([P, ND], F32, name=f"ln_{n}", tag=f"ln_{n}")
            nc.sync.dma_start(out=t, in_=ln_d[n].rearrange("(a p) -> p a", p=P))
            ln_sb[n] = t

        bq96, bk96, vbias, bo_sb = {}, {}, {}, {}
        for b in ("sp", "tp", "cx"):
            t = const.tile([HD, H], F32, name=f"bq96_{b}", tag=f"bq96_{b}")
            nc.sync.dma_start(out=t, in_=bq_d[b].rearrange("(h k) -> k h", k=HD))
            bq96[b] = t
            t = const.tile([HD, H], F32, name=f"bk96_{b}", tag=f"bk96_{b}")
            nc.sync.dma_start(out=t, in_=bk_d[b].rearrange("(h k) -> k h", k=HD))
            bk96[b] = t
            t = const.tile([P, D], F32, name=f"vb_{b}", tag=f"vb_{b}")
            nc.gpsimd.dma_start(out=t, in_=_bcast_ap(bv_d[b]))
            vbias[b] = t
            t = const.tile([P, ND], F32, name=f"bo_{b}", tag=f"bo_{b}")
            nc.sync.dma_start(out=t, in_=bo_d[b].rearrange("(a p) -> p a", p=P))
            bo_sb[b] = t

        mB1_sb = const.tile([P, NF], F32, name="mB1_sb", tag="mB1")
        nc.sync.dma_start(out=mB1_sb, in_=mB1_d.rearrange("(a p) -> p a", p=P))
        mB2_sb = const.tile([P, ND], F32, name="mB2_sb", tag="mB2")
        nc.sync.dma_start(out=mB2_sb, in_=mB2_d.rearrange("(a p) -> p a", p=P))

        # ---- body helpers ----

        def ln_stats(src):
            """src: [P, ND, T] bf16. Returns (meanb, rstdb) [P, T] bf16."""
            ps_m = pst("ps_m")
            ps_s = pst("ps_s")
            for a in range(ND):
                sq = tmp.tile([P, T], BF, name="sq", tag="tmp")
                nc.scalar.square(sq, src[:, a, :])
                nc.tensor.matmul(
                    ps_m[0:1, :], ones_bf, src[:, a, :],
                    start=(a == 0), stop=(a == ND - 1),
                )
                nc.tensor.matmul(
                    ps_s[0:1, :], ones_bf, sq,
                    start=(a == 0), stop=(a == ND - 1),
                )
            mrow = rows.tile([1, T], F32, name="mrow", tag="rows")
            nc.vector.tensor_scalar_mul(mrow, ps_m[0:1, :], 1.0 / D)
            srow = rows.tile([1, T], F32, name="srow", tag="rows")
            nc.vector.tensor_scalar_mul(srow, ps_s[0:1, :], 1.0 / D)
            trow = rows.tile([1, T], F32, name="trow", tag="rows")
            nc.vector.tensor_mul(trow, mrow, mrow)
            # var = E[x^2] - m^2 ; rstd = exp(-0.5*ln(var+eps))
            nc.vector.tensor_sub(srow, srow, trow)
            urow = rows.tile([1, T], F32, name="urow", tag="rows")
            nc.scalar.activation(urow, srow, AF.Ln, bias=epsc[0:1, :])
            rrow = rows.tile([1, T], BF, name="rrow", tag="rows")
            nc.scalar.activation(rrow, urow, AF.Exp, scale=-0.5)
            mrow_bf = rows.tile([1, T], BF, name="mrow_bf", tag="rows")
            nc.vector.tensor_copy(mrow_bf, mrow)

            meanb = big.tile([P, T], BF, name="meanb", tag="meanb", bufs=2)
            nc.gpsimd.partition_broadcast(meanb, mrow_bf)
            rstdb = big.tile([P, T], BF, name="rstdb", tag="rstdb", bufs=2)
            nc.gpsimd.partition_broadcast(rstdb, rrow)
            return meanb, rstdb

        def ln_apply(src, meanb, rstdb, outs):
            """outs: list of (dst [P, ND, T] bf16, gamma_sb, beta_sb)."""
            for a in range(ND):
                xc = tmp.tile([P, T], BF, name="xc", tag="tmp")
                nc.vector.tensor_sub(xc, src[:, a, :], meanb)
                nc.vector.tensor_mul(xc, xc, rstdb)
                for dst, g_sb, b_sb in outs:
                    nc.scalar.activation(
                        dst[:, a, :], xc, AF.Identity,
                        bias=b_sb[:, a : a + 1], scale=g_sb[:, a : a + 1],
                    )

        def gemm_head(src, wqk_sb, qki, bias96, h, dst96, use_act=False):
            """dst96[0:HD, :] = (W[:, head h cols].T @ src) + bias."""
            pq = pst("pq")
            for a in range(ND):
                nc.tensor.matmul(
                    pq[0:HD, :], wqk_sb[:, qki, h, a, :], src[:, a, :],
                    start=(a == 0), stop=(a == ND - 1),
                )
            if use_act:
                nc.scalar.activation(
                    dst96[0:HD, :], pq[0:HD, :], AF.Identity,
                    bias=bias96[:, h : h + 1],
                )
            else:
                nc.vector.tensor_scalar_add(
                    dst96[0:HD, :], pq[0:HD, :], bias96[:, h : h + 1]
                )

        def gemm_v_token(src, wv_sb, vbias_bc, Vt):
            """Vt: [P, NT, H, HD+1] token-major V with trailing ones column."""
            nc.vector.memset(Vt[:, :, :, HD], 1.0)
            for half, n in ((0, 512), (1, 256)):
                pvs = [pst(f"pv{t}") for t in range(NT)]
                for a in range(ND):
                    for t in range(NT):
                        nc.tensor.matmul(
                            pvs[t][:, 0:n],
                            src[:, a, t * P : (t + 1) * P],
                            wv_sb[:, a, half * 512 : half * 512 + n],
                            start=(a == 0), stop=(a == ND - 1),
                        )
                for t in range(NT):
                    if half == 0:
                        nc.vector.tensor_add(
                            Vt[:, t, 0:5, 0:HD],
                            pvs[t][:, 0:480].rearrange("p (h k) -> p h k", k=HD),
                            vbias_bc[:, 0:480].rearrange("p (h k) -> p h k", k=HD),
                        )
                        nc.vector.tensor_add(
                            Vt[:, t, 5, 0:32],
                            pvs[t][:, 480:512],
                            vbias_bc[:, 480:512],
                        )
                    else:
                        nc.vector.tensor_add(
                            Vt[:, t, 5, 32:HD],
                            pvs[t][:, 0:64],
                            vbias_bc[:, 512:576],
                        )
                        nc.vector.tensor_add(
                            Vt[:, t, 6:8, 0:HD],
                            pvs[t][:, 64:256].rearrange("p (h k) -> p h k", k=HD),
                            vbias_bc[:, 576:768].rearrange("p (h k) -> p h k", k=HD),
                        )

        def attn_head_core(qh, kh, Vt, attnT, causal, h):
            """Scores, exp, PV, deferred-softmax normalization for one head."""
            ets = []
            for jc in range(NT):
                i0 = jc * P if causal else 0
                pS = pst("pS")
                nc.tensor.matmul(
                    pS[:, 0 : T - i0],
                    kh[0:HD, jc * P : (jc + 1) * P],
                    qh[0:HD, i0:T],
                    start=True, stop=True,
                )
                et = ex.tile([P, T], BF, name="et", tag="ex")
                nc.scalar.activation(
                    et[:, i0:T], pS[:, 0 : T - i0], AF.Exp, scale=SCALE
                )
                if causal:
                    nc.gpsimd.tensor_mul(
                        et[:, i0 : i0 + P], et[:, i0 : i0 + P], diag_sb
                    )
                ets.append(et)
            pa = pst("pa")
            for jc in range(NT):
                i0 = jc * P if causal else 0
                # jc=0 covers the full psum row (start lazily zeroes the whole
                # 2KB zero region); later jc's accumulate only their causal
                # suffix [jc*P:T].
                nc.tensor.matmul(
                    pa[0 : HD + 1, i0:T], Vt[:, jc, h, :], ets[jc][:, i0:T],
                    start=(jc == 0), stop=(jc == NT - 1),
                )
            srow = rows.tile([HD + 1, T], BF, name="sumrow", tag="srow", bufs=3)
            nc.vector.reciprocal(srow[HD : HD + 1, :], pa[HD : HD + 1, :])
            # Replicating SBUF->SBUF DMA: broadcast the reciprocal row
            # (partition HD) to all partitions via a stride-0 middle dim.
            s = srow[HD : HD + 1, :]
            rbc = tmp.tile([P, T], BF, name="rbc", tag="tmp")
            nc.sync.dma_start(
                out=rbc,
                in_=bass.AP(
                    tensor=s.tensor, offset=s.offset,
                    ap=[list(s.ap[0]), [0, P], list(s.ap[-1])],
                ),
            )
            nc.vector.tensor_mul(
                attnT[0:HD, h, :], pa[0:HD, :], rbc[0:HD, :]
            )

        def attn_branch(src_q, src_k, Vt, attnT, causal, wqk_sb, bq, bk):
            """Per-head q/k projection software-pipelined with attention."""
            qs, ks = [None] * H, [None] * H
            for h in range(H):
                qs[h] = qk.tile([P, T], BF, name="qh", tag="qk")
                ks[h] = qk.tile([P, T], BF, name="kh", tag="qk")
                gemm_head(src_q, wqk_sb, 0, bq, h, qs[h], use_act=True)
                gemm_head(src_k, wqk_sb, 1, bk, h, ks[h], use_act=False)
                if h > 0:
                    attn_head_core(qs[h - 1], ks[h - 1], Vt, attnT, causal, h - 1)
                    qs[h - 1] = ks[h - 1] = None
            attn_head_core(qs[H - 1], ks[H - 1], Vt, attnT, causal, H - 1)

        def out_proj(attnT, wo_sb, bias_sb, dst, residual=None):
            for e in range(ND):
                po = pst("po")
                for h in range(H):
                    nc.tensor.matmul(
                        po, wo_sb[:, e, h, :], attnT[0:HD, h, :],
                        start=(h == 0), stop=(h == H - 1),
                    )
                if residual is None:
                    nc.vector.tensor_scalar_add(
                        dst[:, e, :], po, bias_sb[:, e : e + 1]
                    )
                else:
                    nc.vector.scalar_tensor_tensor(
                        dst[:, e, :], po, bias_sb[:, e : e + 1],
                        residual[:, e, :], ALU.add, ALU.add,
                    )

        def load_wqk(b):
            t = wqkp.tile([P, 2, H, ND, HD], BF, name=f"wqk_{b}", tag="wqk")
            nc.sync.dma_start(
                out=t,
                in_=wqk_d[b].rearrange("p (q h a k) -> p q h a k", q=2, h=H, a=ND),
            )
            return t

        def load_wv(b):
            t = wvp.tile([P, ND, D], BF, name=f"wv_{b}", tag="wv")
            nc.sync.dma_start(
                out=t, in_=wv_d[b].rearrange("p (a e) -> p a e", a=ND)
            )
            return t

        def load_wo(b):
            t = wop.tile([HD, ND, H, P], BF, name=f"wo_{b}", tag="wo")
            nc.sync.dma_start(
                out=t, in_=wo_d[b].rearrange("k (e h ec) -> k e h ec", e=ND, h=H)
            )
            return t

        def mlp(xn3, x1T, outT_sb):
            ph2 = [
                ps.tile([P, T], F32, name=f"h2_{e}", tag="ps", bufs=8)
                for e in range(ND)
            ]
            for c in range(NF // FCC):
                w1t = w1p.tile([P, FCC, ND, P], BF, name="w1t", tag="w1")
                nc.sync.dma_start(
                    out=w1t,
                    in_=mW1_d.rearrange("p (f a e) -> p f a e", f=NF, a=ND)[
                        :, c * FCC : (c + 1) * FCC, :, :
                    ],
                )
                w2t = w2p.tile([P, FCC, D], BF, name="w2t", tag="w2")
                nc.sync.dma_start(
                    out=w2t,
                    in_=mW2_d.rearrange("p (f e) -> p f e", f=NF)[
                        :, c * FCC : (c + 1) * FCC, :
                    ],
                )
                for j in range(FCC):
                    fc = c * FCC + j
                    ph1 = pst("ph1")
                    for a in range(ND):
                        nc.tensor.matmul(
                            ph1, w1t[:, j, a, :], xn3[:, a, :],
                            start=(a == 0), stop=(a == ND - 1),
                        )
                    yt = tmp.tile([P, T], BF, name="yt", tag="tmp")
                    nc.scalar.activation(
                        yt, ph1, AF.Gelu, bias=mB1_sb[:, fc : fc + 1]
                    )
                    for e in range(ND):
                        nc.tensor.matmul(
                            ph2[e], w2t[:, j, e * P : (e + 1) * P], yt,
                            start=(fc == 0), stop=(fc == NF - 1),
                        )
            for e in range(ND):
                nc.vector.scalar_tensor_tensor(
                    outT_sb[:, e, :], ph2[e], mB2_sb[:, e : e + 1],
                    x1T[:, e, :], ALU.add, ALU.add,
                )

        def _mark(phase):
            PHASE_MARKS.append((phase, int(nc.get_next_instruction_name()[2:])))

        def body():
            _mark("load_x")
            xT_sb = big.tile([P, ND, T], BF, name="xT_sb", tag="xT")
            nc.sync.dma_start(
                out=xT_sb, in_=xT_d.rearrange("(a p) t -> p a t", p=P)
            )
            _mark("ln0")
            meanb, rstdb = ln_stats(xT_sb)
            xn_s = big.tile([P, ND, T], BF, name="xn_s", tag="xn_s")
            xn_t = big.tile([P, ND, T], BF, name="xn_t", tag="xn_t")
            ln_apply(
                xT_sb, meanb, rstdb,
                [
                    (xn_s, ln_sb["g_s"], ln_sb["b_s"]),
                    (xn_t, ln_sb["g_t"], ln_sb["b_t"]),
                ],
            )

            # --- spatial branch (temporal V is emitted early for overlap) ---
            _mark("sp_v")
            wv_sp = load_wv("sp")
            Vt = big.tile([P, NT, H, HD + 1], BF, name="Vt_s", tag="Vt", bufs=2)
            gemm_v_token(xn_s, wv_sp, vbias["sp"], Vt)
            _mark("tp_v")
            wv_tp = load_wv("tp")
            Vt2 = big.tile([P, NT, H, HD + 1], BF, name="Vt_t", tag="Vt", bufs=2)
            gemm_v_token(xn_t, wv_tp, vbias["tp"], Vt2)
            attnT = big.tile([P, H, T], BF, name="attnT_s", tag="attnT", bufs=2)
            _mark("sp_attn")
            wqk_sp = load_wqk("sp")
            attn_branch(xn_s, xn_s, Vt, attnT, False, wqk_sp, bq96["sp"], bk96["sp"])
            _mark("sp_oproj")
            wo_sp = load_wo("sp")
            soT = big.tile([P, ND, T], BF, name="soT", tag="soT")
            out_proj(attnT, wo_sp, bo_sb["sp"], soT)

            # --- temporal branch ---
            attnT2 = big.tile([P, H, T], BF, name="attnT_t", tag="attnT", bufs=2)
            _mark("tp_attn")
            wqk_tp = load_wqk("tp")
            attn_branch(xn_t, xn_t, Vt2, attnT2, True, wqk_tp, bq96["tp"], bk96["tp"])
            _mark("tp_oproj")
            wo_tp = load_wo("tp")
            toT = big.tile([P, ND, T], BF, name="toT", tag="toT")
            out_proj(attnT2, wo_tp, bo_sb["tp"], toT, residual=xn_t)

            # --- cross attention ---
            _mark("cx_v")
            wv_cx = load_wv("cx")
            Vt3 = big.tile([P, NT, H, HD + 1], BF, name="Vt_c", tag="Vt", bufs=2)
            gemm_v_token(toT, wv_cx, vbias["cx"], Vt3)
            attnT3 = big.tile([P, H, T], BF, name="attnT_c", tag="attnT", bufs=2)
            _mark("cx_attn")
            wqk_cx = load_wqk("cx")
            attn_branch(soT, toT, Vt3, attnT3, False, wqk_cx, bq96["cx"], bk96["cx"])
            _mark("cx_oproj")
            wo_cx = load_wo("cx")
            x1T = big.tile([P, ND, T], BF, name="x1T", tag="x1T")
            out_proj(attnT3, wo_cx, bo_sb["cx"], x1T, residual=xT_sb)

            # --- MLP ---
            _mark("ln3")
            meanb3, rstdb3 = ln_stats(x1T)
            xn3 = big.tile([P, ND, T], BF, name="xn3", tag="xn_s")
            ln_apply(x1T, meanb3, rstdb3, [(xn3, ln_sb["g_m"], ln_sb["b_m"])])
            outT_sb = big.tile([P, ND, T], BF, name="outT_sb", tag="toT")
            _mark("mlp")
            mlp(xn3, x1T, outT_sb)
            nc.sync.dma_start(
                out=outT_d.rearrange("(a p) t -> p a t", p=P), in_=outT_sb
            )

        if repeat == 1:
            body()
        else:
            with tc.For_i(0, repeat, 1):
                body()

    nc.compile()
    return nc


def _route(inputs):
    """Top-1 expert indices per sample, computed exactly as the reference
    (jax on CPU, f32) — softmax is monotonic so argmax of logits suffices."""
    import jax
    import jax.numpy as jnp

    cpu = jax.devices("cpu")[0]
    with jax.default_device(cpu):
        x = jnp.asarray(inputs["x"])
        h = jax.nn.gelu(
            x.mean(1) @ jnp.asarray(inputs["router_w1"]).T
            + jnp.asarray(inputs["router_b1"]),
            approximate=False,
        )
        logits = (
            h @ jnp.asarray(inputs["router_w2"]).T + jnp.asarray(inputs["router_b2"])
        )
        logits = np.asarray(logits)
    K = logits.shape[1] // 2
    idx_s = np.argmax(logits[:, :K], axis=-1)
    idx_t = np.argmax(logits[:, K:], axis=-1)
    return idx_s, idx_t


_cache = {}


def _get_nc(repeat=1):
    key = ("nc", repeat)
    if key not in _cache:
        _cache[key] = build(repeat=repeat)
    return _cache[key]


def _f(a):
    return np.ascontiguousarray(np.asarray(a), dtype=np.float32)


def _bf(a):
    return np.ascontiguousarray(np.asarray(a, dtype=np.float32).astype(NPBF))


def _pack_qk_pair(wqT, wkT):
    # wqT/wkT: [D, D] = W^T columns (d, e); e = h*HD+k.
    # -> [P, 2*H*ND*HD] so the whole q/k weight pair is one contiguous DMA.
    arr = np.stack([np.asarray(wqT), np.asarray(wkT)])  # [2, D, D]
    arr = arr.reshape(2, ND, P, H, HD).transpose(2, 0, 3, 1, 4)
    return _bf(arr.reshape(P, 2 * H * ND * HD))


def _pack_v(wT):
    # [D, D] (d, e) -> [P, ND*D]
    return _bf(np.asarray(wT).reshape(ND, P, D).transpose(1, 0, 2).reshape(P, ND * D))


def _pack_wo(w):
    # w: [D, D] (e, d) -> W^T[d, e], d = h*HD+k -> [HD, ND*H*P]
    wt = np.asarray(w).T.reshape(H, HD, ND, P)
    return _bf(wt.transpose(1, 2, 0, 3).reshape(HD, ND * H * P))


def _pack_w1(w1):
    # w1: [DFF, D] -> W1^T [D, DFF] -> [P, NF*ND*P]
    w1t = np.asarray(w1).T.reshape(ND, P, NF, P)
    return _bf(w1t.transpose(1, 2, 0, 3).reshape(P, NF * ND * P))


def _pack_w2(w2):
    # w2: [D, DFF] -> W2^T [DFF, D] -> [P, NF*D]
    w2t = np.asarray(w2).T.reshape(NF, P, D)
    return _bf(w2t.transpose(1, 0, 2).reshape(P, NF * D))


def make_in_maps(inputs):
    idx_s, idx_t = _route(inputs)
    diag = np.triu(np.ones((P, P), dtype=np.float32))  # 1 where p <= q
    cWqkvT = np.asarray(inputs["cross_wqkv"]).astype(np.float32).T
    cb = _f(inputs["cross_bqkv"])
    shared = dict(
        diag=_bf(diag),
        g_s=_f(inputs["norm_s_g"]),
        b_s=_f(inputs["norm_s_b"]),
        g_t=_f(inputs["norm_t_g"]),
        b_t=_f(inputs["norm_t_b"]),
        g_m=_f(inputs["norm_mlp_g"]),
        b_m=_f(inputs["norm_mlp_b"]),
        cxWqk=_pack_qk_pair(cWqkvT[:, 0:D], cWqkvT[:, D : 2 * D]),
        cxWv=_pack_v(cWqkvT[:, 2 * D : 3 * D]),
        cxWo=_pack_wo(np.asarray(inputs["cross_wo"])),
        cxBq=cb[0:D],
        cxBk=cb[D : 2 * D],
        cxBv=cb[2 * D : 3 * D],
        cxBo=_f(inputs["cross_bo"]),
        mW1=_pack_w1(np.asarray(inputs["mlp_w1"])),
        mB1=_f(inputs["mlp_b1"]),
        mW2=_pack_w2(np.asarray(inputs["mlp_w2"])),
        mB2=_f(inputs["mlp_b2"]),
    )
    x = np.asarray(inputs["x"])
    in_maps = []
    for b in range(NCORES):
        s = int(idx_s[b])
        t = int(idx_t[b])
        m = dict(shared)
        m["xT"] = _bf(x[b].T)
        spWqkvT = np.asarray(inputs["sp_wqkv"])[s].astype(np.float32).T
        spb = _f(np.asarray(inputs["sp_bqkv"])[s])
        m["spWqk"] = _pack_qk_pair(spWqkvT[:, 0:D], spWqkvT[:, D : 2 * D])
        m["spWv"] = _pack_v(spWqkvT[:, 2 * D : 3 * D])
        m["spWo"] = _pack_wo(np.asarray(inputs["sp_wo"])[s])
        m["spBq"] = spb[0:D]
        m["spBk"] = spb[D : 2 * D]
        m["spBv"] = spb[2 * D : 3 * D]
        m["spBo"] = _f(np.asarray(inputs["sp_bo"])[s])
        m["tpWqk"] = _pack_qk_pair(
            np.asarray(inputs["tp_wq"])[t].astype(np.float32).T,
            np.asarray(inputs["tp_wk"])[t].astype(np.float32).T,
        )
        m["tpWv"] = _pack_v(np.asarray(inputs["tp_wv"])[t].astype(np.float32).T)
        m["tpWo"] = _pack_wo(np.asarray(inputs["tp_wo"])[t])
        m["tpBq"] = _f(np.asarray(inputs["tp_bq"])[t])
        m["tpBk"] = _f(np.asarray(inputs["tp_bk"])[t])
        m["tpBv"] = _f(np.asarray(inputs["tp_bv"])[t])
        m["tpBo"] = _f(np.asarray(inputs["tp_bo"])[t])
        in_maps.append(m)
    return in_maps


def kernel(**inputs) -> np.ndarray:
    repeat = int(os.environ.get("KREPEAT", "1"))
    nc = _get_nc(repeat=repeat)
    in_maps = make_in_maps(inputs)
    res = bass_utils.run_bass_kernel_spmd(nc, in_maps, core_ids=list(range(NCORES)))
    out = np.stack(
        [
            np.ascontiguousarray(
                np.asarray(res.results[b]["outT"], dtype=np.float32).T
            )
            for b in range(NCORES)
        ]
    )
    return out
